# revision 74
# baseline (speedup 1.0000x reference)
"""Multi-head causal self-attention on 8 Trainium2 NeuronCores.

Problem: x[4,2048,1024] fp32, Wq/Wk/Wv/Wo[1024,1024], H=16 heads, head_dim=64,
causal mask, attention_mask all-ones (per spec fill=ones -> no-op).

Sharding (hybrid data/tensor parallel):
  core c -> batch b = c//2, head-half hh = c%2 (8 heads = 512 features).
  Each core: Q/K/V projections with column-sliced W (Megatron column
  parallel), attention for its 8 heads, o_proj with row-sliced Wo
  (row parallel) producing a partial [2048,1024] output. The host sums
  the two partials per batch (the "all-reduce") and stacks batches.

Device kernel (per core), fp32 PSUM accumulate throughout:
  Q/K/V projections run in fp8-e4m3 DoubleRow (0.5 cyc/row, 2x bf16 PE
  rate) with a 3-term error-compensated split: x and W are decomposed
  host-side into hi+lo e4m3 pairs and the kernel computes
  x_hi@W_hi + x_lo@W_hi + x_hi@W_lo (dropping only the ~2^-9 lo@lo
  term), pairing two 128-row k-tiles per DoubleRow instruction -> 12
  DRs replace 8 bf16 matmuls (0.75x cycles) at better-than-bf16
  accuracy. Weights are pre-scaled by powers of 2 to dodge e4m3
  subnormals; the net 2^13 score scale is folded into exp's free
  scale constant, Wv's 2^5 into Wo host-side.
  QT/KT land in transposed [feat, seq] layout, scores are computed
  transposed (scoresT[k,q] = KT_blk.T @ QT) so exp runs on ScalarE
  straight out of PSUM. Within each 2-key-block unit the later block
  sits at columns [0:512] (kbl swap) so the two causal spans merge
  into one exp instruction with minimal stale-column waste. AV runs
  with the *feature* dim moving (out[q,f] = E_blk.T @ [V|1]); the
  ones-column yields sumexp per query in the same pass. Normalization
  is a per-partition scalar multiply fused into the PSUM->SBUF bf16
  eviction; the [feat, seq] layout o_proj needs is restored by XBAR
  DMA transposes (off the PE/DVE critical path) except for the last
  chunk, which uses PE transposes to keep the drain latency short.
  Causal handling: block-skip fully-masked key blocks, one 128x128
  triangular mask multiply (on DVE) for diagonal blocks.
"""

import numpy as np
import ml_dtypes

_BF16 = ml_dtypes.bfloat16
_F8 = ml_dtypes.float8_e4m3
_B, _S, _D = 4, 2048, 1024
_NCORES = 8
_HPC = 8   # heads per core
_FT = 4    # 128-wide feature tiles per core (= head pairs)
_DT = 8    # 128-wide tiles of D
_SB = 16   # 128-wide seq blocks
_QC = 4    # 512-wide seq chunks

_cache = {}


def _build_nc(opts=None):
    opts = opts or {}
    import concourse.bacc as bacc
    import concourse.mybir as mybir
    import concourse.tile as tile
    from concourse.bass import ts

    f32 = mybir.dt.float32
    bf16 = mybir.dt.bfloat16
    fp8 = mybir.dt.float8e4
    DR = mybir.MatmulPerfMode.DoubleRow
    Exp = mybir.ActivationFunctionType.Exp

    fp8_proj = opts.get("fp8_proj", True)
    fp8_oproj = opts.get("fp8_oproj", True) and fp8_proj
    # hybrid: fp8 DoubleRow o_proj for chunks 0-2 (ctx hi/lo split on DVE
    # mid-pipeline, off the critical path); chunk 3 + drain stay bf16
    hyb8 = opts.get("hyb8", False) and fp8_proj and not fp8_oproj
    # host pre-scales: Wq' = Wq/8 * 2^8, Wk' = Wk * 2^5 -> scores carry 2^13,
    # folded into exp's free scale constant. Wv' = Wv * 2^5, Wo' = Wo / 2^5
    # (bf16 o_proj) or Wo * 2^5 with a host-side /1024 on y (fp8 o_proj).
    exp_scale = 2.0 ** -13 if fp8_proj else 1.0

    nc = bacc.Bacc("TRN2", target_bir_lowering=False, debug=False)

    if fp8_proj:
        xth = nc.dram_tensor("xth", [_D, _S], fp8, kind="ExternalInput")
        xtl = nc.dram_tensor("xtl", [_D, _S], fp8, kind="ExternalInput")
        wqh = nc.dram_tensor("wqh", [_D, 512], fp8, kind="ExternalInput")
        wql = nc.dram_tensor("wql", [_D, 512], fp8, kind="ExternalInput")
        wkh = nc.dram_tensor("wkh", [_D, 512], fp8, kind="ExternalInput")
        wkl = nc.dram_tensor("wkl", [_D, 512], fp8, kind="ExternalInput")
        wvh = nc.dram_tensor("wvh", [_D, 512], fp8, kind="ExternalInput")
        wvl = nc.dram_tensor("wvl", [_D, 512], fp8, kind="ExternalInput")
    else:
        xt = nc.dram_tensor("xt", [_D, _S], bf16, kind="ExternalInput")   # x[b].T
        wq = nc.dram_tensor("wq", [_D, 512], bf16, kind="ExternalInput")  # pre-scaled 1/8
        wk = nc.dram_tensor("wk", [_D, 512], bf16, kind="ExternalInput")
        wv = nc.dram_tensor("wv", [_D, 512], bf16, kind="ExternalInput")
    if fp8_oproj:
        woh = nc.dram_tensor("woh", [512, _D], fp8, kind="ExternalInput")
        wol = nc.dram_tensor("wol", [512, _D], fp8, kind="ExternalInput")
    else:
        wo = nc.dram_tensor("wo", [512, _D], bf16, kind="ExternalInput")
        if hyb8:
            woh = nc.dram_tensor("woh", [512, _D], fp8, kind="ExternalInput")
            wol = nc.dram_tensor("wol", [512, _D], fp8, kind="ExternalInput")
    ydt = bf16 if opts.get("y_bf16", True) else f32
    y = nc.dram_tensor("y", [_S, _D], ydt, kind="ExternalOutput")

    with tile.TileContext(nc) as tc:
        with (
            tc.tile_pool(name="const", bufs=1) as constp,
            tc.tile_pool(name="win", bufs=1) as wp,
            tc.tile_pool(name="acts", bufs=1) as actp,
            tc.tile_pool(name="ex", bufs=opts.get("ex_bufs", 22)) as exp_pool,
            tc.tile_pool(name="ev", bufs=opts.get("ev_bufs", 6)) as ev_pool,
            tc.tile_pool(name="nrm", bufs=opts.get("nrm_bufs", 12)) as nrm_pool,
            tc.tile_pool(name="ps_proj", bufs=opts.get("proj_bufs", 2), space="PSUM") as ps_proj,
            tc.tile_pool(name="ps_big", bufs=2, space="PSUM") as ps_big,
            tc.tile_pool(name="ps_av", bufs=opts.get("av_bufs", 2), space="PSUM") as ps_av,
        ):
            # ---- input loads, split and ordered by first use so the first
            # matmuls start after ~1/8 of the bytes land ---------------------
            if fp8_oproj:
                wohs = wp.tile([128, _FT, _D], fp8, name="wohs", tag="wohs")
                wols = wp.tile([128, _FT, _D], fp8, name="wols", tag="wols")
            else:
                wos = wp.tile([128, _FT, _D], bf16, name="wos", tag="wos")
                if hyb8:
                    wohs = wp.tile([128, _FT, _D], fp8, name="wohs", tag="wohs")
                    wols = wp.tile([128, _FT, _D], fp8, name="wols", tag="wols")
            if fp8_proj:
                # chunk-major so each 512-col chunk load is SBUF-contiguous
                # (HWDGE-eligible); matmuls slice [:, sc, dt-pair, :]
                xhs = wp.tile([128, _QC, _DT, 512], fp8, name="xhs", tag="xhs")
                xls = wp.tile([128, _QC, _DT, 512], fp8, name="xls", tag="xls")
                wqhs = wp.tile([128, _DT, 512], fp8, name="wqhs", tag="wqhs")
                wqls = wp.tile([128, _DT, 512], fp8, name="wqls", tag="wqls")
                wkhs = wp.tile([128, _DT, 512], fp8, name="wkhs", tag="wkhs")
                wkls = wp.tile([128, _DT, 512], fp8, name="wkls", tag="wkls")
                wvhs = wp.tile([128, _DT, 512], fp8, name="wvhs", tag="wvhs")
                wvls = wp.tile([128, _DT, 512], fp8, name="wvls", tag="wvls")
                xh_r = xth[:].rearrange("(dt p) (sc s) -> p sc dt s", p=128, s=512)
                xl_r = xtl[:].rearrange("(dt p) (sc s) -> p sc dt s", p=128, s=512)
                w_pairs = [
                    (wqhs, wqh), (wqls, wql), (wkhs, wkh), (wkls, wkl),
                    (wvhs, wvh), (wvls, wvl),
                ]
                w_rs = {id(t): d[:].rearrange("(dt p) n -> p dt n", p=128)
                        for t, d in w_pairs}
                if opts.get("fp8_fast", True) and opts.get("q3_start", False):
                    # sync(SP) DMAs ride slow SWDGE queues; pool rides HWDGE.
                    # Startup-critical chunk-0 x goes on the (idle) ACT queue,
                    # Wk leads the sync queue, weights stream on pool.
                    for d in range(2):
                        dd = slice(4 * d, 4 * d + 4)
                        nc.scalar.dma_start(xhs[:, 0, dd], xh_r[:, 0, dd])
                        nc.gpsimd.dma_start(wqhs[:, dd], w_rs[id(wqhs)][:, dd])
                        nc.sync.dma_start(wkhs[:, dd], w_rs[id(wkhs)][:, dd])
                    nc.scalar.dma_start(xls[:, 0], xl_r[:, 0])
                    nc.gpsimd.dma_start(wqls[:], w_rs[id(wqls)][:])
                    nc.sync.dma_start(wkls[:], w_rs[id(wkls)][:])
                    nc.gpsimd.dma_start(wvhs[:], w_rs[id(wvhs)][:])
                    nc.gpsimd.dma_start(wvls[:], w_rs[id(wvls)][:])
                    for sc in range(1, _QC):
                        nc.sync.dma_start(xhs[:, sc], xh_r[:, sc])
                        nc.sync.dma_start(xls[:, sc], xl_r[:, sc])
                    nc.gpsimd.dma_start(
                        wos[:], wo[:].rearrange("(ft p) n -> p ft n", p=128))
                elif opts.get("fp8_fast", True):
                    # sync queue: x chunk0 then Wk then x chunks 1-3;
                    # pool queue: Wq, Wv, Wo. First Q-proj starts once
                    # xh0+wqh land (~2us), K follows, V trickles later.
                    x0n = opts.get("x0_chunks", 4)
                    wk_pool = opts.get("wk_pool", False)
                    for d in range(x0n):
                        dd = slice(_DT // x0n * d, _DT // x0n * (d + 1))
                        nc.sync.dma_start(xhs[:, 0, dd], xh_r[:, 0, dd])
                    for d in range(2):
                        dd = slice(4 * d, 4 * d + 4)
                        nc.gpsimd.dma_start(wqhs[:, dd], w_rs[id(wqhs)][:, dd])
                        if wk_pool:
                            nc.gpsimd.dma_start(wkhs[:, dd], w_rs[id(wkhs)][:, dd])
                    nc.sync.dma_start(xls[:, 0], xl_r[:, 0])
                    nc.gpsimd.dma_start(wqls[:], w_rs[id(wqls)][:])
                    if wk_pool:
                        nc.gpsimd.dma_start(wkls[:], w_rs[id(wkls)][:])
                    else:
                        for d in range(2):
                            dd = slice(4 * d, 4 * d + 4)
                            nc.sync.dma_start(wkhs[:, dd], w_rs[id(wkhs)][:, dd])
                        nc.sync.dma_start(wkls[:], w_rs[id(wkls)][:])
                    nc.gpsimd.dma_start(wvhs[:], w_rs[id(wvhs)][:])
                    nc.gpsimd.dma_start(wvls[:], w_rs[id(wvls)][:])
                    for sc in range(1, _QC):
                        nc.sync.dma_start(xhs[:, sc], xh_r[:, sc])
                        nc.sync.dma_start(xls[:, sc], xl_r[:, sc])
                    if fp8_oproj:
                        nc.gpsimd.dma_start(
                            wohs[:], woh[:].rearrange("(ft p) n -> p ft n", p=128))
                        nc.gpsimd.dma_start(
                            wols[:], wol[:].rearrange("(ft p) n -> p ft n", p=128))
                    else:
                        nc.gpsimd.dma_start(
                            wos[:], wo[:].rearrange("(ft p) n -> p ft n", p=128))
                        if hyb8:
                            nc.gpsimd.dma_start(
                                wohs[:], woh[:].rearrange("(ft p) n -> p ft n", p=128))
                            nc.gpsimd.dma_start(
                                wols[:], wol[:].rearrange("(ft p) n -> p ft n", p=128))
                else:
                    for dts in (slice(0, 1), slice(1, 2), slice(2, 4), slice(4, 8)):
                        nc.sync.dma_start(wvhs[:, dts], w_rs[id(wvhs)][:, dts])
                        nc.sync.dma_start(wvls[:, dts], w_rs[id(wvls)][:, dts])
                        nc.sync.dma_start(xhs[:, 0, dts], xh_r[:, 0, dts])
                        nc.sync.dma_start(xls[:, 0, dts], xl_r[:, 0, dts])
                    for dh in range(2):
                        dts = slice(dh * 4, dh * 4 + 4)
                        nc.sync.dma_start(wqhs[:, dts], w_rs[id(wqhs)][:, dts])
                        nc.sync.dma_start(wqls[:, dts], w_rs[id(wqls)][:, dts])
                        nc.sync.dma_start(wkhs[:, dts], w_rs[id(wkhs)][:, dts])
                        nc.sync.dma_start(wkls[:, dts], w_rs[id(wkls)][:, dts])
                    for sc in range(1, _QC):
                        nc.sync.dma_start(xhs[:, sc], xh_r[:, sc])
                        nc.sync.dma_start(xls[:, sc], xl_r[:, sc])
                    if fp8_oproj:
                        nc.sync.dma_start(
                            wohs[:], woh[:].rearrange("(ft p) n -> p ft n", p=128))
                        nc.sync.dma_start(
                            wols[:], wol[:].rearrange("(ft p) n -> p ft n", p=128))
                    else:
                        nc.sync.dma_start(
                            wos[:], wo[:].rearrange("(ft p) n -> p ft n", p=128))
            else:
                xts = wp.tile([128, _DT, _S], bf16, name="xts", tag="xts")
                wqs = wp.tile([128, _DT, 512], bf16, name="wqs", tag="wqs")
                wks = wp.tile([128, _DT, 512], bf16, name="wks", tag="wks")
                wvs = wp.tile([128, _DT, 512], bf16, name="wvs", tag="wvs")
                xt_r = xt[:].rearrange("(dt p) s -> p dt s", p=128)
                wq_r = wq[:].rearrange("(dt p) n -> p dt n", p=128)
                wk_r = wk[:].rearrange("(dt p) n -> p dt n", p=128)
                wv_r = wv[:].rearrange("(dt p) n -> p dt n", p=128)
            if fp8_proj:
                pass
            elif opts.get("fast_start", False):
                # dt-interleaved so the first Q-projection pass starts after
                # ~2KB/partition and streams as chunks land
                for dt in range(_DT):
                    nc.sync.dma_start(wqs[:, dt:dt + 1], wq_r[:, dt:dt + 1])
                    nc.sync.dma_start(
                        xts[:, dt:dt + 1, 0:512], xt_r[:, dt:dt + 1, 0:512])
                for dh in range(2):
                    dts = slice(dh * 4, dh * 4 + 4)
                    nc.sync.dma_start(wks[:, dts], wk_r[:, dts])
                # wv and x chunk 1 trickle together: V-proj and chunk-1 QK
                # projections both start consuming around the same time
                for dh in range(4):
                    dts = slice(2 * dh, 2 * dh + 2)
                    nc.sync.dma_start(wvs[:, dts], wv_r[:, dts])
                    nc.sync.dma_start(
                        xts[:, dts, 512:1024], xt_r[:, dts, 512:1024])
                nc.sync.dma_start(wos[:], wo[:].rearrange("(ft p) n -> p ft n", p=128))
                for sc in range(2, _QC):
                    nc.sync.dma_start(
                        xts[:, :, ts(sc, 512)], xt_r[:, :, ts(sc, 512)])
            elif opts.get("dma2q", False):
                # weights ride the Pool-triggered queue, activations the SP
                # queue: the two DMA streams drain in parallel so inputs land
                # in about half the serial time
                for dts in (slice(0, 1), slice(1, 2), slice(2, 4), slice(4, 8)):
                    nc.gpsimd.dma_start(wvs[:, dts], wv_r[:, dts])
                    nc.sync.dma_start(xts[:, dts, 0:512], xt_r[:, dts, 0:512])
                for dh in range(2):
                    dts = slice(dh * 4, dh * 4 + 4)
                    nc.gpsimd.dma_start(wqs[:, dts], wq_r[:, dts])
                    nc.gpsimd.dma_start(wks[:, dts], wk_r[:, dts])
                for sc in range(1, _QC):
                    nc.sync.dma_start(
                        xts[:, :, ts(sc, 512)], xt_r[:, :, ts(sc, 512)])
                nc.gpsimd.dma_start(wos[:], wo[:].rearrange("(ft p) n -> p ft n", p=128))
            else:
                for dts in (slice(0, 1), slice(1, 2), slice(2, 4), slice(4, 8)):
                    nc.sync.dma_start(wvs[:, dts], wv_r[:, dts])
                    nc.sync.dma_start(xts[:, dts, 0:512], xt_r[:, dts, 0:512])
                for dh in range(2):
                    dts = slice(dh * 4, dh * 4 + 4)
                    nc.sync.dma_start(wqs[:, dts], wq_r[:, dts])
                    nc.sync.dma_start(wks[:, dts], wk_r[:, dts])
                for sc in range(1, _QC):
                    nc.sync.dma_start(
                        xts[:, :, ts(sc, 512)], xt_r[:, :, ts(sc, 512)])
                nc.sync.dma_start(wos[:], wo[:].rearrange("(ft p) n -> p ft n", p=128))

            # PE p-state warmup: dependency-free matmuls on a memset tile keep
            # the tensor engine continuously busy through the initial DMA wait
            # so the first real matmuls run at full clock (3us ramp)
            warmup_n = opts.get("warmup_n", 0)
            if warmup_n:
                wut = constp.tile([128, 128], bf16, name="wut", tag="wut")
                nc.gpsimd.memset(wut[:], 0.0)
                wups = ps_proj.tile([128, 512], f32, tag="psproj", name="wups")
                for i in range(warmup_n):
                    nc.tensor.matmul(
                        wups[:, 0:128], lhsT=wut[:], rhs=wut[:],
                        start=True, stop=True, skip_group_check=True,
                    )

            # causal mask for diagonal 128x128 blocks: keep iff q_rel >= k_rel
            mask0 = constp.tile([128, 128], bf16, name="mask0", tag="mask0")
            nc.gpsimd.memset(mask0[:], 1.0)
            nc.gpsimd.affine_select(
                out=mask0[:], in_=mask0[:],
                compare_op=mybir.AluOpType.is_ge, fill=0.0,
                base=0, channel_multiplier=-1, pattern=[[1, 128]],
            )
            # 128x128 bf16 identity for PE transposes
            ident = constp.tile([128, 128], bf16, name="ident", tag="ident")
            nc.gpsimd.memset(ident[:], 1.0)
            nc.gpsimd.affine_select(
                out=ident[:], in_=ident[:],
                compare_op=mybir.AluOpType.is_equal, fill=0.0,
                base=0, channel_multiplier=-1, pattern=[[1, 128]],
            )

            # V with a ones-column appended per head: [128, sb, head, 64+1]
            vxs = actp.tile([128, _SB, _HPC, 65], bf16, name="vxs", tag="vxs")
            nc.gpsimd.memset(vxs[:, :, :, 64], 1.0)

            qts = [actp.tile([128, _S], bf16, name=f"qt{ft}", tag=f"qt{ft}") for ft in range(_FT)]
            kts = [actp.tile([128, _S], bf16, name=f"kt{ft}", tag=f"kt{ft}") for ft in range(_FT)]
            if fp8_oproj:
                # single tiles so DoubleRow lhsT can pair ft slices via dim1
                ctxh = actp.tile([128, _FT, _S], fp8, name="ctxh", tag="ctxh")
                ctxl = actp.tile([128, _FT, _S], fp8, name="ctxl", tag="ctxl")
            else:
                ctxs = [actp.tile([128, _S], bf16, name=f"ctx{ft}", tag=f"ctx{ft}") for ft in range(_FT)]
                if hyb8:
                    # fp8 hi/lo shadows of ctx for chunks 0-2 (cols 0:1536)
                    ctxh = actp.tile([128, _FT, 1536], fp8, name="ctxh", tag="ctxh")
                    ctxl = actp.tile([128, _FT, 1536], fp8, name="ctxl", tag="ctxl")

            diag64 = opts.get("diag64", False)

            _eng = {"dve": nc.vector, "pool": nc.gpsimd}
            ctx_eng = _eng[opts.get("ctx_eng", "dve")]

            if opts.get("proj_cp_scalar", False):
                proj_cp = nc.scalar.copy
            elif opts.get("proj_cp_pool", False):
                proj_cp = nc.gpsimd.tensor_copy
            else:
                proj_cp = nc.vector.tensor_copy

            def proj_v(sb):
                ps = ps_proj.tile([128, 512], f32, tag="psproj", name="psv")
                if fp8_proj:
                    # 3-term compensated fp8: x_hi@W_hi + x_lo@W_hi + x_hi@W_lo,
                    # DoubleRow pairs two 128-row k-tiles per instruction at
                    # 0.5 cyc/row -> 12 DRs = 0.75x the bf16 cycle count
                    if opts.get("terms_xl_last", False):
                        prods = [(xhs, wvhs), (xhs, wvls), (xls, wvhs)]
                    else:
                        prods = [(xhs, wvhs), (xls, wvhs), (xhs, wvls)]
                    n = 0
                    sc0, j0 = divmod(sb, 4)
                    for xsrc, wsrc in prods:
                        for d in range(_DT // 2):
                            dd = slice(2 * d, 2 * d + 2)
                            nc.tensor.matmul(
                                ps[:], lhsT=xsrc[:, sc0, dd, ts(j0, 128)],
                                rhs=wsrc[:, dd, :], perf_mode=DR,
                                start=(n == 0), stop=(n == 11),
                            )
                            n += 1
                else:
                    for dt in range(_DT):
                        nc.tensor.matmul(
                            ps[:], lhsT=xts[:, dt, ts(sb, 128)], rhs=wvs[:, dt, :],
                            start=(dt == 0), stop=(dt == _DT - 1),
                        )
                proj_cp(
                    vxs[:, sb, :, 0:64], ps[:].rearrange("p (h d) -> p h d", h=_HPC)
                )

            def proj_qk1(sc, ft, which):
                ps = ps_proj.tile([128, 512], f32, tag="psproj", name="psqk")
                dst = qts[ft] if which == 0 else kts[ft]
                if fp8_proj:
                    wh, wl = ((wqhs, wqls) if which == 0 else (wkhs, wkls))
                    if opts.get("terms_xl_last", False):
                        prods = [(xhs, wh), (xhs, wl), (xls, wh)]
                    else:
                        prods = [(xhs, wh), (xls, wh), (xhs, wl)]
                    n = 0
                    for xsrc, wsrc in prods:
                        for d in range(_DT // 2):
                            dd = slice(2 * d, 2 * d + 2)
                            nc.tensor.matmul(
                                ps[:], lhsT=wsrc[:, dd, ts(ft, 128)],
                                rhs=xsrc[:, sc, dd, :], perf_mode=DR,
                                start=(n == 0), stop=(n == 11),
                            )
                            n += 1
                else:
                    wsrc = wqs if which == 0 else wks
                    for dt in range(_DT):
                        nc.tensor.matmul(
                            ps[:], lhsT=wsrc[:, dt, ts(ft, 128)],
                            rhs=xts[:, dt, ts(sc, 512)],
                            start=(dt == 0), stop=(dt == _DT - 1),
                        )
                proj_cp(dst[:, ts(sc, 512)], ps[:])

            def chunk_tasks(sc):
                # projection work needed before attention chunk sc runs
                t = [(lambda sb=sb: proj_v(sb)) for sb in range(4 * sc, 4 * sc + 4)]
                t += [(lambda sc=sc, ft=ft, w=w: proj_qk1(sc, ft, w))
                      for ft in range(_FT) for w in range(2)]
                return t

            start_pending = []
            if fp8_proj and opts.get("fp8_fast", True):
                # only head-pair 0's Q/K needed before the first score matmul;
                # the rest trickles in between the first units, V interleaved
                # so vxs blocks land before the first AV consumes them
                proj_qk1(0, 0, 0)
                proj_qk1(0, 0, 1)
                orders = {
                    0: [(1, 0), (1, 1), ("v", 0), ("v", 1),
                        (2, 0), (2, 1), ("v", 2), ("v", 3), (3, 0), (3, 1)],
                    1: [(1, 0), (1, 1), (2, 0), (2, 1), ("v", 0), ("v", 1),
                        (3, 0), (3, 1), ("v", 2), ("v", 3)],
                    3: [(1, 0), (1, 1), (2, 0), (2, 1), (3, 0), (3, 1),
                        ("v", 0), ("v", 1), ("v", 2), ("v", 3)],
                    2: [("v", 0), (1, 0), (1, 1), ("v", 1), (2, 0), (2, 1),
                        ("v", 2), (3, 0), (3, 1), ("v", 3)],
                }
                order = orders[opts.get("start_order", 0)]
                for a, b in order:
                    if a == "v":
                        start_pending.append(lambda sb=b: proj_v(sb))
                    else:
                        start_pending.append(
                            lambda ft=a, w=b: proj_qk1(0, ft, w))
            elif (not fp8_proj) and opts.get("fast_start", False):
                # only head-pair 0's Q/K needed before the first score matmul;
                # everything else trickles in between the first units
                proj_qk1(0, 0, 0)
                proj_qk1(0, 0, 1)
                start_pending += [(lambda ft=ft, w=w: proj_qk1(0, ft, w))
                                  for ft in range(1, _FT) for w in range(2)]
                start_pending += [(lambda sb=sb: proj_v(sb)) for sb in range(4)]
            else:
                for t in chunk_tasks(0):
                    t()

            ocnt = [0]

            def oproj_mms(ps, qb, nn2):
                if fp8_oproj or (hyb8 and qb < 4 * (_QC - 1)):
                    prods = [(ctxh, wohs), (ctxl, wohs), (ctxh, wols)]
                    n = 0
                    for csrc, wsrc in prods:
                        for fpi in range(_FT // 2):
                            ff = slice(2 * fpi, 2 * fpi + 2)
                            nc.tensor.matmul(
                                ps[:], lhsT=csrc[:, ff, ts(qb, 128)],
                                rhs=wsrc[:, ff, ts(nn2, 512)], perf_mode=DR,
                                start=(n == 0), stop=(n == 5),
                            )
                            n += 1
                else:
                    for ft in range(_FT):
                        nc.tensor.matmul(
                            ps[:], lhsT=ctxs[ft][:, ts(qb, 128)],
                            rhs=wos[:, ft, ts(nn2, 512)],
                            start=(ft == 0), stop=(ft == _FT - 1),
                        )

            def oproj(qb, nn2, pool=None, tag=None, spread=False):
                # one 512-wide half of output row-block qb
                ps = (pool or ps_proj).tile([128, 512], f32, tag=tag or "psproj", name="pso")
                oproj_mms(ps, qb, nn2)
                ev = ev_pool.tile([128, 512], ydt, tag="ev", name="ev")
                ocnt[0] += 1
                cp = (nc.scalar.copy if (spread and ocnt[0] % 2)
                      else nc.vector.tensor_copy)
                cp(ev[:], ps[:])
                eng = nc.gpsimd if (spread and ocnt[0] % 2) else nc.sync
                eng.dma_start(y[:][ts(qb, 128), ts(nn2, 512)], ev[:])

            def oproj_pair(qb, pool_tags):
                # both halves of one output row-block -> one ev -> one DMA;
                # the two evictions split across DVE and Pool so the tail
                # chains overlap
                ev = ev_pool.tile([128, 1024], ydt, tag="ev2", name="ev2")
                for nn2 in range(2):
                    pool, tag = pool_tags[nn2 % len(pool_tags)]
                    ps = (pool or ps_proj).tile([128, 512], f32, tag=tag or "psproj", name="pso")
                    oproj_mms(ps, qb, nn2)
                    cp = (nc.scalar.copy
                          if (nn2 == 1 and opts.get("pair_spread", True))
                          else nc.vector.tensor_copy)
                    cp(ev[:, ts(nn2, 512)], ps[:])
                ocnt[0] += 1
                eng = (nc.gpsimd if (opts.get("pair_dma_alt", False) and ocnt[0] % 2)
                       else nc.sync)
                eng.dma_start(y[:][ts(qb, 128), :], ev[:])

            pending = []
            soft = []  # o_proj work: deferred to the end flush (measured fastest)
            tq = []    # deferred ctx transpose work (PE) so PE never waits DVE
            av_split = opts.get("av_split", True)

            kbl_swap = opts.get("kbl_swap", True)
            # column base of key-block kbl inside scps/exs tiles. Swapped
            # (kb1 at cols [0:512], kb0 at [512:1024]) the two causal-valid
            # spans merge with only soff0 waste cols instead of soff1.
            kcb = (lambda kbl: 512 - 512 * kbl) if kbl_swap else (lambda kbl: 512 * kbl)

            def emit_scores(qc, hp, kbg):
                # scoresT[k, q] = KT_blk.T @ QT_chunk; heads of the pair
                # interleave (row groups 0-63 / 64-127 run concurrently)
                scps = [ps_big.tile([128, 1024], f32, name=f"scp{_i}", tag="scp") for _i in range(2)]
                exs = [exp_pool.tile([128, 1024], bf16, name=f"ex{_i}", tag="ex") for _i in range(2)]
                soffs = []
                for kbl in range(2):
                    kb = 2 * kbg + kbl
                    off = (kb - 4 * qc) * 128
                    soffs.append(off if off > 0 else 0)
                # head-major order: scpA finishes ASAP so its exp (and
                # the next kb-group's A-scores) start earlier
                for h01 in range(2):
                    pb = 64 * h01
                    for kbl in range(2):
                        kb = 2 * kbg + kbl
                        soff = soffs[kbl]
                        cb = kcb(kbl)
                        if diag64 and kb - 4 * qc >= 0 and not kbl_swap:
                            # diagonal block: split the key dim so the
                            # strictly-masked 64x64 corner (keys 64-127 vs
                            # the first 64 queries) is never computed. The
                            # first matmul's start zeroes the whole bank,
                            # so the skipped corner reads as exp(0)=1 and
                            # is zeroed by the mask multiply.
                            nc.tensor.matmul(
                                scps[h01][0:64, kbl * 512 + soff:(kbl + 1) * 512],
                                lhsT=kts[hp][pb:pb + 64, kb * 128:kb * 128 + 64],
                                rhs=qts[hp][pb:pb + 64, qc * 512 + soff:(qc + 1) * 512],
                                start=True, stop=False, skip_group_check=True,
                            )
                            nc.tensor.matmul(
                                scps[h01][64:128, kbl * 512 + soff + 64:(kbl + 1) * 512],
                                lhsT=kts[hp][pb:pb + 64, kb * 128 + 64:kb * 128 + 128],
                                rhs=qts[hp][pb:pb + 64, qc * 512 + soff + 64:(qc + 1) * 512],
                                start=False, stop=True, skip_group_check=True,
                            )
                        else:
                            nc.tensor.matmul(
                                scps[h01][:, cb + soff:cb + 512],
                                lhsT=kts[hp][pb:pb + 64, ts(kb, 128)],
                                rhs=qts[hp][pb:pb + 64, qc * 512 + soff:(qc + 1) * 512],
                                start=True, stop=True,
                            )
                # with kbl_swap the merged span is [soff1:1024) and only the
                # soff0 cols at [512:512+soff0) are stale waste; without it
                # the span is [soff0:1024) with soff1 waste cols
                lo, waste = (soffs[1], soffs[0]) if kbl_swap else (soffs[0], soffs[1])
                merge = waste <= opts.get("exp_merge_max", 384)
                for h01 in range(2):
                    if waste == 0 or (opts.get("exp_merge", True) and merge):
                        # one instr over the whole span: the waste cols hold
                        # stale psum whose exp lands in exs cols the AV
                        # matmuls never read (block-skipped by causality)
                        nc.scalar.activation(
                            exs[h01][:, lo:1024],
                            scps[h01][:, lo:1024], Exp, scale=exp_scale)
                    else:
                        nc.scalar.activation(
                            exs[h01][:, lo:512],
                            scps[h01][:, lo:512], Exp, scale=exp_scale)
                        nc.scalar.activation(
                            exs[h01][:, 512 + (soffs[0] if kbl_swap else soffs[1]):1024],
                            scps[h01][:, 512 + (soffs[0] if kbl_swap else soffs[1]):1024],
                            Exp, scale=exp_scale)
                return exs

            def emit_av(qc, hp, kbg, exs, av):
                # all diagonal masks first (DVE batches them while PE is
                # still on scores), then the AV matmuls with the feature
                # dim moving: av[q, 128*(4*h01+qs) + f] += E.T @ [V|1]
                maskeng = nc.gpsimd if opts.get("mask_gpsimd", True) else nc.vector
                for kbl in range(2):
                    kb = 2 * kbg + kbl
                    off = (kb - 4 * qc) * 128
                    cb = kcb(kbl)
                    if off >= 0:
                        for h01 in range(2):
                            maskeng.tensor_mul(
                                exs[h01][:, cb + off:cb + off + 128],
                                exs[h01][:, cb + off:cb + off + 128],
                                mask0[:],
                            )
                for h01 in range(2):
                    h = 2 * hp + h01
                    for kbl in range(2):
                        kb = 2 * kbg + kbl
                        off = (kb - 4 * qc) * 128
                        cb = kcb(kbl)
                        for qs in range(4):
                            if qs * 128 < off:
                                continue
                            if av_split:
                                dst = av[h01][:, 128 * qs:128 * qs + 65]
                            else:
                                dst = av[:, 128 * (4 * h01 + qs):128 * (4 * h01 + qs) + 65]
                            # start=True zeroes the WHOLE psum bank on TRN2,
                            # so only the first matmul into each tile starts;
                            # all other regions accumulate onto the zeroed
                            # bank.
                            nc.tensor.matmul(
                                dst,
                                lhsT=exs[h01][:, cb + qs * 128:cb + qs * 128 + 128],
                                rhs=vxs[:, kb, h, 0:65],
                                start=(kb == 0 and qs == 0),
                                stop=(kb == 4 * qc + qs),
                                skip_group_check=True,
                            )

            def emit_norm(qc, hp, av):
                # recips of the 8 sumexp columns, then normalize each
                # [128q, 64f] region into a bf16 [q, f-pair] tile; the PE
                # transpose back to [f, q] is deferred via tq so the PE
                # stream never waits on DVE.
                rt = nrm_pool.tile([128, 8], f32, tag="rt", name="rt")
                if av_split:
                    for h01 in range(2):
                        nc.vector.reciprocal(
                            rt[:, 4 * h01:4 * h01 + 4],
                            av[h01][:].rearrange("p (r c) -> p r c", c=128)[:, :, 64])
                else:
                    nc.vector.reciprocal(
                        rt[:], av[:].rearrange("p (r c) -> p r c", c=128)[:, :, 64])
                for qs in range(4):
                    ctxn = nrm_pool.tile([128, 128], bf16, tag="ctxn", name="ctxn")
                    for h01 in range(2):
                        src = (av[h01][:, 128 * qs:128 * qs + 64] if av_split
                               else av[:, 128 * (4 * h01 + qs):128 * (4 * h01 + qs) + 64])
                        nc.vector.tensor_scalar_mul(
                            ctxn[:, h01 * 64:h01 * 64 + 64],
                            src,
                            rt[:, 4 * h01 + qs:4 * h01 + qs + 1],
                        )
                    tq.append((qc, hp, qs, ctxn))

            dma_tq = opts.get("dma_tq", True) and not fp8_oproj
            tq_dma_engs = ([nc.sync, nc.scalar] if opts.get("tq_dma_act", False)
                           else [nc.sync])

            tq_pe_last = opts.get("tq_pe_last", True)

            def pop_tq(n=None):
                k = len(tq) if n is None else min(n, len(tq))
                for _ in range(k):
                    qc_, hp_, qs_, ctxn = tq.pop(0)
                    cols = slice((4 * qc_ + qs_) * 128, (4 * qc_ + qs_ + 1) * 128)
                    if dma_tq and not (tq_pe_last and qc_ == _QC - 1):
                        # XBAR DMA transpose: off the PE/DVE critical path
                        eng = tq_dma_engs[(4 * qc_ + qs_) % len(tq_dma_engs)]
                        eng.dma_start_transpose(ctxs[hp_][:, cols], ctxn[:])
                        if hyb8 and qc_ < _QC - 1:
                            hi = ctxh[:, hp_, cols]
                            nc.vector.tensor_copy(hi, ctxs[hp_][:, cols])
                            nc.vector.tensor_sub(
                                ctxl[:, hp_, cols], ctxs[hp_][:, cols], hi)
                        continue
                    tp = ps_proj.tile([128, 128], bf16, tag="psproj", name="tp")
                    nc.tensor.transpose(tp[:], ctxn[:], ident[:])
                    if fp8_oproj:
                        hi = ctxh[:, hp_, cols]
                        ctx_eng.tensor_copy(hi, tp[:])
                        ctx_eng.tensor_sub(ctxl[:, hp_, cols], tp[:], hi)
                    else:
                        ctx_eng.tensor_copy(ctxs[hp_][:, cols], tp[:])

            def finish_unit(qc, hp, kbg, exs):
                av = avs_by_hp.get((qc, hp))
                if av is None:
                    if av_split:
                        av = [ps_av.tile([128, 512], f32, name=f"av{_i}", tag="av")
                              for _i in range(2)]
                    else:
                        av = ps_av.tile([128, 1024], f32, name="av", tag="av")
                    avs_by_hp[(qc, hp)] = av
                emit_av(qc, hp, kbg, exs, av)
                if kbg == (4 * qc + 4) // 2 - 1:
                    emit_norm(qc, hp, avs_by_hp.pop((qc, hp)))
                    if hp == _FT - 1:
                        ot = [(qb, nn2) for qb in range(4 * qc, 4 * qc + 4)
                              for nn2 in range(2)]
                        if qc + 1 < _QC:
                            soft.extend(ot)
                        else:
                            flush_final(ot)

            def oproj_fine(qb, pool_tags):
                # last row-block: evict + DMA in small parallel pieces across
                # DVE/ACT engines and both DMA queues so the post-PE tail is
                # one 256-col chain instead of a 1024-col one
                ev = ev_pool.tile([128, 1024], ydt, tag="ev2", name="ev2")
                cps = [nc.vector.tensor_copy, nc.scalar.copy]
                dmas = [nc.sync, nc.gpsimd]
                for nn2 in range(2):
                    pool, tag = pool_tags[nn2 % len(pool_tags)]
                    ps = (pool or ps_proj).tile([128, 512], f32, tag=tag or "psproj", name="pso")
                    oproj_mms(ps, qb, nn2)
                    for half in range(2):
                        c0 = 256 * half
                        cps[half](ev[:, nn2 * 512 + c0:nn2 * 512 + c0 + 256],
                                  ps[:, c0:c0 + 256])
                        dmas[half].dma_start(
                            y[:][ts(qb, 128), nn2 * 512 + c0:nn2 * 512 + c0 + 256],
                            ev[:, nn2 * 512 + c0:nn2 * 512 + c0 + 256])

            def flush_final(otasks):
                # final flush: rotate the finished chunk's o_proj
                # groups across all freed psum pools
                pop_tq()
                rot = [(ps_proj, "psproj"), (ps_big, "scp"), (ps_av, "av")]
                spread = opts.get("tail_spread", False)
                for i, (qb, nn2) in enumerate(soft):
                    pool, tag = rot[i % 3] if opts.get("soft_rot", False) else (None, None)
                    oproj(qb, nn2, pool, tag, spread=spread)
                if opts.get("flush_ft_last", False) and opts.get("pair_final", True):
                    # emit ft0-2 for the first 3 pairs up front: they depend
                    # only on head-pairs 0-2, so they overlap the last
                    # head-pair's norm->transpose latency; the hp3-dependent
                    # ft3 matmuls + evictions trail
                    qbs = sorted({qb for qb, _ in otasks})
                    held = []
                    for i, qb in enumerate(qbs[:3]):
                        ev = ev_pool.tile([128, 1024], ydt, tag="ev2", name="ev2")
                        pss = []
                        for nn2 in range(2):
                            pool, tag = rot[(2 * i + nn2) % 3]
                            ps = pool.tile([128, 512], f32, tag=tag, name="pso")
                            for ft in range(_FT - 1):
                                nc.tensor.matmul(
                                    ps[:], lhsT=ctxs[ft][:, ts(qb, 128)],
                                    rhs=wos[:, ft, ts(nn2, 512)],
                                    start=(ft == 0), stop=False,
                                    skip_group_check=True,
                                )
                            pss.append(ps)
                        held.append((qb, ev, pss))
                    for qb, ev, pss in held:
                        for nn2 in range(2):
                            nc.tensor.matmul(
                                pss[nn2][:], lhsT=ctxs[_FT - 1][:, ts(qb, 128)],
                                rhs=wos[:, _FT - 1, ts(nn2, 512)],
                                start=False, stop=True, skip_group_check=True,
                            )
                            cp = (nc.scalar.copy if nn2 == 1
                                  else nc.vector.tensor_copy)
                            cp(ev[:, ts(nn2, 512)], pss[nn2][:])
                        nc.sync.dma_start(y[:][ts(qb, 128), :], ev[:])
                    oproj_pair(qbs[3], [rot[0], rot[1]])
                elif opts.get("pair_final", True):
                    qbs = sorted({qb for qb, _ in otasks})
                    fine = opts.get("fine_tail", False)
                    for i, qb in enumerate(qbs):
                        pt = [rot[(2 * i) % 3], rot[(2 * i + 1) % 3]]
                        if fine and qb == qbs[-1]:
                            oproj_fine(qb, pt)
                        else:
                            oproj_pair(qb, pt)
                else:
                    for i, (qb, nn2) in enumerate(otasks):
                        pool, tag = rot[i % 3]
                        oproj(qb, nn2, pool, tag, spread=spread)

            # software-pipeline the emission across hp AND qc
            # boundaries: scores of the next kb-group enter the PE
            # stream before AV of the previous, so PE never stalls
            # on exp latency.
            inflight = []
            avs_by_hp = {}
            kbg_ctr = 0
            pending.extend(start_pending)
            even_pop = opts.get("even_pop", True)
            wpop = opts.get("weighted_pop", False)
            for qc in range(_QC):
                pending.extend(chunk_tasks(qc + 1) if qc + 1 < _QC else [])
                kbgs_left = _FT * (2 * qc + 2)
                len0, units0, popped0, u0 = len(pending), kbgs_left, 0, 0
                if wpop:
                    # filler quota proportional to each unit's exp width so
                    # PE has more slack work exactly where ACT runs longest
                    uw = []
                    for kbg in range(2 * qc + 2):
                        s1 = max(0, (2 * kbg + 1 - 4 * qc) * 128)
                        uw.append(64 + 1024 - s1)
                    uw = uw * _FT
                    wtot = sum(uw)
                    wacc = 0.0
                for hp in range(_FT):
                    for kbg in range(2 * qc + 2):
                        # interleave next chunk's projection work; spread
                        # evenly across the whole chunk so late units keep
                        # PE filler while ScalarE catches up on exp
                        if wpop:
                            wacc += uw[hp * (2 * qc + 2) + kbg]
                            want = int(wacc * len0 / wtot + 0.999)
                            npop = min(len(pending), max(0, want - popped0))
                            popped0 += npop
                        elif even_pop:
                            u0 += 1
                            want = (u0 * len0 + units0 - 1) // units0
                            npop = min(len(pending), max(0, want - popped0))
                            popped0 += npop
                        else:
                            npop = -(-len(pending) // kbgs_left) if pending else 0
                        kbgs_left -= 1
                        for _ in range(npop):
                            pending.pop(0)()
                        pop_tq(opts.get("tq_rate", 1))
                        if opts.get("soft_qc3", True):
                            # in the last chunk pop every unit, but hold
                            # back enough o_proj work to fill the final
                            # pipeline-drain phase
                            q2r = opts.get("soft_qc2_rate", 0)
                            q3r = opts.get("soft_qc3_rate", 1)
                            do_soft = soft and (
                                (qc == _QC - 1
                                 and len(soft) > opts.get("soft_keep", 16)
                                 and kbg_ctr % q3r == 0)
                                or (q2r and qc == _QC - 2
                                    and kbg_ctr % q2r == 0)
                                or (not pending
                                    and kbg_ctr % opts.get("soft_rate", 2) == 0))
                        else:
                            do_soft = (soft and not pending
                                       and kbg_ctr % opts.get("soft_rate", 2) == 0)
                        if do_soft and not opts.get("soft_after", False):
                            qb_, nn2_ = soft.pop(0)
                            oproj(qb_, nn2_)
                        kbg_ctr += 1
                        if opts.get("av_first", False):
                            # retire the oldest unit BEFORE emitting new
                            # scores: if scores stall on the exp ring, the
                            # ready AV work would otherwise be stuck behind
                            # them in the in-order PE queue
                            if len(inflight) >= opts.get("depth", 9):
                                q0, h0, k0, e0 = inflight.pop(0)
                                finish_unit(q0, h0, k0, e0)
                            inflight.append((qc, hp, kbg, emit_scores(qc, hp, kbg)))
                        else:
                            inflight.append((qc, hp, kbg, emit_scores(qc, hp, kbg)))
                            if len(inflight) >= opts.get("depth", 9):
                                q0, h0, k0, e0 = inflight.pop(0)
                                finish_unit(q0, h0, k0, e0)
                        if do_soft and opts.get("soft_after", False):
                            qb_, nn2_ = soft.pop(0)
                            oproj(qb_, nn2_)
            for q0, h0, k0, e0 in inflight:
                finish_unit(q0, h0, k0, e0)
                pop_tq(opts.get("tq_rate", 1))
                for _ in range(opts.get("drain_n", 1)):
                    if soft and opts.get("drain_soft", True):
                        qb_, nn2_ = soft.pop(0)
                        oproj(qb_, nn2_)

    nc.compile()
    return nc


# best-measured schedule knobs; _get_nc() with no args uses these
_BEST = {"depth": 11, "soft_keep": 4, "fp8_oproj": False, "dma_tq": True,
         "tq_rate": 1, "mask_gpsimd": False, "soft_rate": 3,
         "tq_pe_last": False, "terms_xl_last": True, "start_order": 1}


def _get_nc(opts=None):
    if opts is None:
        opts = _BEST
    key = tuple(sorted(opts.items()))
    if key not in _cache:
        _cache[key] = _build_nc(opts)
    return _cache[key]


def _hilo(a):
    hi = a.astype(_F8)
    lo = (a - hi.astype(np.float32)).astype(_F8)
    return hi, lo


def _shard(x, Wq, Wk, Wv, Wo, fp8_proj=True, fp8_oproj=True, hyb8=False):
    in_maps = []
    if fp8_proj:
        # scale weights to unit-ish std so e4m3 doesn't hit subnormals;
        # score scale 2^13 is folded into exp's scale const, Wv's 2^5 is
        # folded out of Wo (bf16) or out of y host-side (fp8 o_proj).
        wq_s = Wq * np.float32(0.125 * 256.0)
        wk_s = Wk * np.float32(32.0)
        wv_s = Wv * np.float32(32.0)
        wo_s = Wo * np.float32(32.0 if fp8_oproj else 1.0 / 32.0)
        wo_f8 = Wo * np.float32(32.0) if hyb8 else None
    for c in range(_NCORES):
        b, hh = divmod(c, 2)
        cols = slice(512 * hh, 512 * hh + 512)
        if fp8_proj:
            xh, xl = _hilo(np.ascontiguousarray(x[b].T))
            wqh, wql = _hilo(np.ascontiguousarray(wq_s[:, cols]))
            wkh, wkl = _hilo(np.ascontiguousarray(wk_s[:, cols]))
            wvh, wvl = _hilo(np.ascontiguousarray(wv_s[:, cols]))
            m = {
                "xth": xh, "xtl": xl,
                "wqh": wqh, "wql": wql,
                "wkh": wkh, "wkl": wkl,
                "wvh": wvh, "wvl": wvl,
            }
            if fp8_oproj:
                m["woh"], m["wol"] = _hilo(np.ascontiguousarray(wo_s[cols, :]))
            else:
                m["wo"] = np.ascontiguousarray(wo_s[cols, :]).astype(_BF16)
                if hyb8:
                    m["woh"], m["wol"] = _hilo(
                        np.ascontiguousarray(wo_f8[cols, :]))
            in_maps.append(m)
        else:
            in_maps.append({
                "xt": np.ascontiguousarray(x[b].T).astype(_BF16),
                "wq": (Wq[:, cols] * np.float32(0.125)).astype(_BF16),
                "wk": np.ascontiguousarray(Wk[:, cols]).astype(_BF16),
                "wv": np.ascontiguousarray(Wv[:, cols]).astype(_BF16),
                "wo": np.ascontiguousarray(Wo[cols, :]).astype(_BF16),
            })
    return in_maps


def _run(inputs, trace=False):
    from concourse import bass_utils

    x = np.asarray(inputs["x"], dtype=np.float32)
    Wq = np.asarray(inputs["Wq"], dtype=np.float32)
    Wk = np.asarray(inputs["Wk"], dtype=np.float32)
    Wv = np.asarray(inputs["Wv"], dtype=np.float32)
    Wo = np.asarray(inputs["Wo"], dtype=np.float32)
    # attention_mask is all-ones by problem spec (fill=ones) -> no-op.

    nc = _get_nc()
    fp8o = _BEST.get("fp8_oproj", True)
    hyb8 = _BEST.get("hyb8", False) and not fp8o
    res = bass_utils.run_bass_kernel_spmd(
        nc, _shard(x, Wq, Wk, Wv, Wo, fp8_proj=True, fp8_oproj=fp8o, hyb8=hyb8),
        core_ids=list(range(_NCORES)), trace=trace
    )
    # fp8 o_proj carries ctx*32 @ Wo*32 -> y is 1024x the true output;
    # hybrid mode only for query rows 0:1536 (chunks 0-2)
    ysc = np.float32(1.0 / 1024.0) if fp8o else np.float32(1.0)
    ys = []
    for r in res.results:
        yv = r["y"].astype(np.float32) * ysc
        if hyb8:
            yv[:1536] *= np.float32(1.0 / 1024.0)
        ys.append(yv)
    out = np.stack([ys[2 * b] + ys[2 * b + 1] for b in range(_B)])
    return out, res


def kernel(**inputs):
    return _run(inputs)[0]

